# revision 34
# baseline (speedup 1.0000x reference)
"""Trainium2 Bass kernel for nn_ActorAction (moe_routing).

Computation (see reference):
  option_embed = embed_table[option]              [B, 64]
  all_state    = concat([state, option_embed])    [B, 576]
  cls_X = MLP_relu(all_state; Wx1,bx1,Wx2,bx2)    [B, 256]
  cls_Y = MLP_relu(all_state; Wy1,by1,Wy2,by2)    [B, 256]
  out_X = cls_X @ noise_lib_X                     [B, 256]
  out_Y[b] = cls_Y[b] @ noise_lib_Y[option[b]]    [B, 256]

Strategy: CLASS-sharded routing. Host ranks the 64 classes by count and
forms 8 rank-groups; group j's 8 classes go one-per-core into slot j,
so every core holds 8 whole classes and only needs THEIR noise_lib_Y
rows (1/8 of NY -> 1.05MB vs 8.4MB replicated). The Bass program is
shared SPMD: slot capacities c_j = max count in rank group j are
compile-time constants; per-core inputs (xt columns, ny slot payload)
differ. MLPs run feature-major (weights stationary, samples streamed);
routing is ONE psum-wide matmul pair per slot (c_j <= 128 rows), 16
matmuls total instead of 128 32-row tile_position matmuls. Outputs are
written bf16 to halve output DMA. Input DMAs are split and ordered by
first consumer; a stream of tiny dummy matmuls warms the PE (HAM
un-throttle) before real data lands.
"""
import os
from contextlib import ExitStack

import numpy as np
import ml_dtypes

import concourse.bacc as bacc
import concourse.mybir as mybir
import concourse.tile as tile
from concourse.bass_utils import run_bass_kernel_spmd

F32 = mybir.dt.float32
F32R = mybir.dt.float32r
BF16 = mybir.dt.bfloat16
AFT = mybir.ActivationFunctionType

# problem dims (hardcoded per spec)
B, FEAT, EMB, HID, NCLS = 4096, 512, 64, 1024, 64
LIB = 256          # LIB_X == LIB_Y
OUTJ = 256
NCORES = 8
D_IN = FEAT + EMB          # 576
KO1 = 5                    # ceil(576/128) K-blocks for layer 1
D_PAD = KO1 * 128          # 640
KO2 = HID // 128           # 8
N_WARMUP = 34              # dummy matmuls (N=128) to warm PE during loads

_DT_MAP = {"f32": F32, "f32r": F32R, "bf16": BF16}
_NP_MAP = {"f32": np.float32, "f32r": np.float32, "bf16": ml_dtypes.bfloat16}
DT_A_NAME = os.environ.get("KDT_A", "bf16")    # MLP weights/acts + NX path
DT_NY_NAME = os.environ.get("KDT_NY", "bf16")  # noise_lib_Y + cls_Y path
# ko4 (embed block) has only 64 live K rows: run two mo-chains' ko4 as
# concurrent K=64 matmuls in the two row-halves of the PE array
PAIR_KO4 = os.environ.get("KPAIR", "1") == "1"


def _round_up(a, b):
    return (a + b - 1) // b * b


def _plan(option):
    opt = np.asarray(option).astype(np.int64).ravel()
    assert opt.shape[0] == B
    g = np.bincount(opt, minlength=NCLS)
    order = np.argsort(opt, kind="stable")
    starts = np.concatenate([[0], np.cumsum(g)])
    # pseudo-classes: (class_id, sample idx array), each <= 128 samples
    pieces = []
    for m in range(NCLS):
        idx = order[starts[m]:starts[m + 1]]
        if len(idx) == 0:
            pieces.append((m, idx))
        for o in range(0, len(idx), 128):
            pieces.append((m, idx[o:o + 128]))
    pieces.sort(key=lambda t: -len(t[1]))
    while len(pieces) % NCORES:
        pieces.append((pieces[-1][0], np.empty(0, np.int64)))
    nslots = len(pieces) // NCORES

    caps = []                      # per-slot capacity (compile-time)
    sstart = []                    # per-slot column start
    cls_of = np.zeros((nslots, NCORES), np.int64)  # class in (slot, core)
    core_of = np.empty(B, np.int64)
    col_of = np.empty(B, np.int64)
    row_of = np.empty(B, np.int64)
    s = 0
    for j in range(nslots):
        grp = pieces[NCORES * j:NCORES * (j + 1)]
        cap = max(4, _round_up(len(grp[0][1]), 4))
        for c, (m, idx) in enumerate(grp):
            cls_of[j, c] = m
            if len(idx):
                core_of[idx] = c
                col_of[idx] = s + np.arange(len(idx))
                row_of[idx] = np.arange(len(idx))
        sstart.append(s)
        caps.append(cap)
        s += cap
    SU_pad = s
    rmax = max(caps)

    # column chunks (psum free dim <= 512), boundaries at slot starts
    chunks = []
    c0 = 0
    for j in range(nslots):
        if sstart[j] + caps[j] - c0 > 512:
            chunks.append((c0, sstart[j]))
            c0 = sstart[j]
    chunks.append((c0, SU_pad))
    assert all(c1 - c0 <= 512 for c0, c1 in chunks)
    assert rmax <= 128

    return dict(opt=opt, caps=caps, sstart=sstart, nslots=nslots,
                SU_pad=SU_pad, rmax=rmax, chunks=chunks, cls_of=cls_of,
                core_of=core_of, col_of=col_of, row_of=row_of,
                has_bias=True)


_NC_CACHE = {}


def _build_nc(plan):
    DT_A = _DT_MAP[DT_A_NAME]
    DT_NY = _DT_MAP[DT_NY_NAME]
    SU_pad = plan["SU_pad"]
    chunks = plan["chunks"]
    caps = plan["caps"]
    sstart = plan["sstart"]
    nslots = plan["nslots"]
    rmax = plan["rmax"]
    has_bias = plan["has_bias"]

    key = (tuple(caps), tuple(chunks), has_bias, PAIR_KO4,
           DT_A_NAME, DT_NY_NAME)
    if key in _NC_CACHE:
        return _NC_CACHE[key]

    c_w1 = KO1 * HID
    c_w2 = KO2 * LIB
    c_bx = c_w1 + c_w2                 # X-branch blob columns
    NY_COLS = nslots * 2 * OUTJ

    ch0 = chunks[0][1]
    c_xt_a = KO1 * ch0
    c_xt_b = KO1 * SU_pad - c_xt_a
    c_mo = KO1 * 128           # one mo-block of W1

    nc = bacc.Bacc()
    xt_a_d = nc.dram_tensor("xt_a", [128, c_xt_a], DT_A, kind="ExternalInput")
    xt_b_d = (nc.dram_tensor("xt_b", [128, c_xt_b], DT_A, kind="ExternalInput")
              if c_xt_b else None)
    w1y_d = nc.dram_tensor("w1y", [128, c_w1], DT_A, kind="ExternalInput")
    # w1y piece boundaries (in mo blocks): mo0 | mo1 | mo2-3 | mo4-7
    w1y_pieces = [(0, 1), (1, 2), (2, 4), (4, KO2)]
    w2y_d = nc.dram_tensor("w2y", [128, c_w2], DT_A, kind="ExternalInput")
    blobx_d = nc.dram_tensor("blobx", [128, c_bx], DT_A, kind="ExternalInput")
    blobx_pieces = [(0, 4 * c_mo), (4 * c_mo, c_bx)]  # w1x mo0-3 | rest
    bias_d = (nc.dram_tensor("bias", [128, 20], F32, kind="ExternalInput")
              if has_bias else None)
    ny_d = nc.dram_tensor("ny", [128, NY_COLS], DT_NY, kind="ExternalInput")
    outx_d = nc.dram_tensor("outx", [128, 2 * SU_pad], BF16,
                            kind="ExternalOutput")
    outy_d = nc.dram_tensor("outy", [rmax, nslots * OUTJ], BF16,
                            kind="ExternalOutput")

    with tile.TileContext(nc) as tc, ExitStack() as ctx:
        const = ctx.enter_context(tc.tile_pool(name="const", bufs=1))
        act = ctx.enter_context(tc.tile_pool(name="act", bufs=1))
        hpool = ctx.enter_context(tc.tile_pool(name="hpool", bufs=2))
        mlp_ps = ctx.enter_context(tc.tile_pool(name="mlp_ps", bufs=5, space="PSUM"))
        rt_ps = ctx.enter_context(tc.tile_pool(name="rt_ps", bufs=3, space="PSUM"))

        # input DMAs: few coarse dma_starts, ordered by first consumer.
        # Each dma_start costs ~0.65us of serialized trigger time on the
        # sync sequencer, and all inputs stream through ONE HW queue in
        # trigger order at ~300GB/s, first data ~1.5us after trigger #1.
        # So the order below is tuned to the PE's weight consumption.
        w1_tiles = {"y": [None] * KO2, "x": None}
        w1y_sbs = {}
        for lo, hi in w1y_pieces:
            t = const.tile([128, (hi - lo) * c_mo], DT_A, tag=f"w1y{lo}",
                           name=f"w1y{lo}")
            w1y_sbs[lo] = t
            v = t.rearrange("p (mo ko m) -> p mo ko m", mo=hi - lo, ko=KO1)
            for mo in range(lo, hi):
                w1_tiles["y"][mo] = v[:, mo - lo]
        nc.sync.dma_start(w1y_sbs[0][:], w1y_d[:, 0:c_mo])
        xt_a_sb = const.tile([128, c_xt_a], DT_A, tag="xta", name="xta")
        nc.sync.dma_start(xt_a_sb[:], xt_a_d[:])
        xt_a_v = xt_a_sb.rearrange("p (ko b) -> p ko b", ko=KO1)
        for lo, hi in w1y_pieces[1:]:
            # emitted after xt_a: each piece arrives just ahead of its
            # mo chain's start (the stream is the binding resource)
            nc.sync.dma_start(w1y_sbs[lo][:], w1y_d[:, lo * c_mo:hi * c_mo])
        if c_xt_b:
            xt_b_sb = const.tile([128, c_xt_b], DT_A)
            nc.sync.dma_start(xt_b_sb[:], xt_b_d[:])
        w2y_sb = const.tile([128, c_w2], DT_A)
        nc.sync.dma_start(w2y_sb[:], w2y_d[:])
        if has_bias:
            bias_sb = const.tile([128, 20], F32)
            nc.sync.dma_start(bias_sb[:], bias_d[:])
        ny_sb = const.tile([128, NY_COLS], DT_NY)
        nc.sync.dma_start(ny_sb[:], ny_d[:])
        ny_v = ny_sb.rearrange("p (s ko j) -> p s ko j", s=nslots, ko=2)
        blobx_sb = const.tile([128, c_bx], DT_A)
        for lo, hi in blobx_pieces:
            nc.sync.dma_start(blobx_sb[:, lo:hi], blobx_d[:, lo:hi])
        w1_tiles["x"] = blobx_sb[:, 0:c_w1].rearrange(
            "p (mo ko m) -> p mo ko m", mo=KO2, ko=KO1)
        w2x_sb = blobx_sb[:, c_w1:c_w1 + c_w2]

        # PE warmup: dummy matmuls on an uninitialized tile (values are
        # irrelevant, the psum result is never read) keep the PE busy from
        # right after the preamble so HAM un-throttles before real work.
        warm_sb = const.tile([128, 128], BF16)
        nc.any.memset(warm_sb[:], 0)
        wups = rt_ps.tile([40, 128], F32, tag="rt", name="rt_ps_t")
        for _ in range(N_WARMUP):
            nc.tensor.matmul(wups[:], lhsT=warm_sb[:, :40], rhs=warm_sb[:],
                             start=True, stop=True)

        def xtv(ci, ko, cw):
            if ci == 1:
                return xt_b_sb.rearrange("p (ko b) -> p ko b", ko=KO1)[:, ko, :cw]
            return xt_a_v[:, ko, :cw]

        def w1v(br, mo, ko, msl):
            if br == "y":
                return w1_tiles["y"][mo][:, ko, msl]
            return w1_tiles["x"][:, mo, ko, msl]

        w2_v = {"y": w2y_sb.rearrange("p (ko m) -> p ko m", ko=KO2),
                "x": w2x_sb.rearrange("p (ko m) -> p ko m", ko=KO2)}
        # bias cols: b1y[0:8] b2y[8:10] b1x[10:18] b2x[18:20]
        bcol = {"y": (0, 8), "x": (10, 18)}

        outy_sb = act.tile([128, nslots, OUTJ], BF16, tag="outy")
        outy_dv = outy_d.rearrange("p (s j) -> p s j", s=nslots)
        outxT = act.tile([128, 2, SU_pad], BF16, tag="outxT")
        outx_dv = outx_d.rearrange("p (jo b) -> p jo b", jo=2)
        clsy = act.tile([128, 2, SU_pad], DT_NY, tag="clsy", name="clsy")
        CH_MAX = max(c1 - c0 for c0, c1 in chunks)

        def post_op(idx, out, ps, func, bias_ap):
            # alternate ScalarE/VectorE per index: splits the psum-drain
            # load across both engines so neither gates the PE.
            if idx % 2 == 0:
                if not has_bias and func is AFT.Identity:
                    nc.scalar.copy(out, ps)
                else:
                    nc.scalar.activation(out, ps, func,
                                         bias=bias_ap if has_bias else 0.0)
            elif func is AFT.Relu:
                if has_bias:
                    nc.vector.tensor_scalar(out, ps, bias_ap, 0.0,
                                            mybir.AluOpType.add,
                                            mybir.AluOpType.max)
                else:
                    nc.vector.tensor_scalar(out, ps, 0.0, None,
                                            mybir.AluOpType.max)
            elif has_bias:
                nc.vector.tensor_scalar(out, ps, bias_ap, None,
                                        mybir.AluOpType.add)
            else:
                nc.vector.tensor_copy(out, ps)

        def mlp(br, after_chunk=None):
            # chunk-major: both layers of chunk ci complete before ci+1,
            # so per-chunk consumers (routing / outx DMA) start early.
            h_sb = hpool.tile([128, KO2, SU_pad], DT_A, tag="h", name=f"h_{br}")
            b1o, b2o = bcol[br]
            for ci, (c0, c1) in enumerate(chunks):
                cw = c1 - c0
                if PAIR_KO4:
                    for t in range(KO2 // 2):
                        moa, mob = 2 * t, 2 * t + 1
                        pss = []
                        for mo in (moa, mob):
                            ps = mlp_ps.tile([128, CH_MAX], F32, tag="mlp",
                                             name="mlp_ps_t")[:, :cw]
                            pss.append(ps)
                            for ko in range(KO1 - 1):
                                nc.tensor.matmul(
                                    ps, lhsT=w1v(br, mo, ko, slice(0, 128)),
                                    rhs=xtv(ci, ko, cw),
                                    start=(ko == 0), stop=False)
                        # ko4: both mo chains' embed blocks concurrently in
                        # the two row-halves of the PE (weights for mob are
                        # packed into rows 64:128 of moa's ko4 block)
                        wa = w1v(br, moa, KO1 - 1, slice(0, 128))
                        xr = xtv(ci, KO1 - 1, cw)
                        for hi, ps in enumerate(pss):
                            h0 = 64 * hi
                            nc.tensor.matmul(
                                ps, lhsT=wa[h0:h0 + 64, :],
                                rhs=xr[h0:h0 + 64, :],
                                start=False, stop=True,
                                tile_position=(h0, 0))
                        for hi, mo in enumerate((moa, mob)):
                            post_op(mo, h_sb[:, mo, c0:c1], pss[hi], AFT.Relu,
                                    bias_sb[:, b1o + mo:b1o + mo + 1]
                                    if has_bias else None)
                else:
                    for mo in range(KO2):
                        ps = mlp_ps.tile([128, CH_MAX], F32, tag="mlp",
                                         name="mlp_ps_t")[:, :cw]
                        for ko in range(KO1):
                            nc.tensor.matmul(
                                ps, lhsT=w1v(br, mo, ko, slice(0, 128)),
                                rhs=xtv(ci, ko, cw),
                                start=(ko == 0), stop=(ko == KO1 - 1))
                        post_op(mo, h_sb[:, mo, c0:c1], ps, AFT.Relu,
                                bias_sb[:, b1o + mo:b1o + mo + 1]
                                if has_bias else None)
                for jo in range(2):
                    ps = mlp_ps.tile([128, CH_MAX], F32, tag="mlp",
                                     name="mlp_ps_t")[:, :cw]
                    for ko in range(KO2):
                        nc.tensor.matmul(
                            ps, lhsT=w2_v[br][:, ko, jo * 128:(jo + 1) * 128],
                            rhs=h_sb[:, ko, c0:c1],
                            start=(ko == 0), stop=(ko == KO2 - 1))
                    tgt = clsy if br == "y" else outxT
                    post_op(jo, tgt[:, jo, c0:c1], ps, AFT.Identity,
                            bias_sb[:, b2o + jo:b2o + jo + 1]
                            if has_bias else None)
                if br == "x":
                    nc.sync.dma_start(outx_dv[:, :, c0:c1],
                                      outxT[:, :, c0:c1])
                elif after_chunk is not None:
                    after_chunk(ci, c1)

        # routing: slot j holds one whole class (per core); out_Y rows =
        # clsY[:, s:s+c].T @ NY[slot j]  -- one full-width matmul pair.
        # Emitted per chunk as soon as that chunk's clsY is ready; copies
        # alternate ScalarE/VectorE so psum drains don't gate the PE.
        rt_done = [0]

        def route_upto(ci, c1):
            last = ci == len(chunks) - 1
            j0 = rt_done[0]
            j = j0
            while j < nslots and (last or sstart[j] + caps[j] <= c1):
                ps = rt_ps.tile([128, OUTJ], F32, tag="rt", name="rt_ps_t")
                for ko in range(2):
                    nc.tensor.matmul(
                        ps[0:caps[j], :],
                        lhsT=clsy[:, ko, sstart[j]:sstart[j] + caps[j]],
                        rhs=ny_v[:, j, ko, :],
                        start=(ko == 0), stop=(ko == 1))
                if j % 2 == 0:
                    nc.scalar.copy(outy_sb[:, j, :], ps[:])
                else:
                    nc.vector.tensor_copy(outy_sb[:, j, :], ps[:])
                j += 1
            rt_done[0] = j
            if j > j0:
                nc.sync.dma_start(outy_dv[:, j0:j, :],
                                  outy_sb[0:rmax, j0:j, :])

        mlp("y", after_chunk=route_upto)
        mlp("x")   # out_X comes straight from the fused MLP2-X (DMA inside)

    nc.compile()
    _NC_CACHE[key] = nc
    return nc


def _prepare_inputs(plan, state, option, embed_table, Wx1, bx1, Wx2, bx2,
                    Wy1, by1, Wy2, by2, noise_lib_X, noise_lib_Y):
    np_a = _NP_MAP[DT_A_NAME]
    np_ny = _NP_MAP[DT_NY_NAME]
    SU_pad = plan["SU_pad"]
    opt = plan["opt"]
    nslots = plan["nslots"]
    core_of, col_of = plan["core_of"], plan["col_of"]
    cls_of = plan["cls_of"]

    state = np.asarray(state, np.float32)
    embed_table = np.asarray(embed_table, np.float32)

    # per-core feature-major inputs
    Xall = np.zeros((NCORES, SU_pad, D_PAD), np.float32)
    Xall[core_of, col_of, :FEAT] = state
    Xall[core_of, col_of, FEAT:D_IN] = embed_table[opt]
    if PAIR_KO4:
        # duplicate embed rows into the ko4 zero-pad so the two row-half
        # K=64 matmuls (mo pair) both see the embed features
        Xall[core_of, col_of, D_IN:D_IN + EMB] = embed_table[opt]
    # [NCORES, 128, KO1, SU_pad]
    xt = Xall.transpose(0, 2, 1).reshape(NCORES, KO1, 128, SU_pad) \
        .transpose(0, 2, 1, 3).astype(np_a)
    ch0 = plan["chunks"][0][1]
    xt_a = np.ascontiguousarray(xt[:, :, :, :ch0]).reshape(NCORES, 128, -1)
    xt_b = np.ascontiguousarray(xt[:, :, :, ch0:]).reshape(NCORES, 128, -1)

    def pack_w1(w):
        # mo-major: [128p, mo, ko, 128] flattened
        w = np.asarray(w, np.float32)
        wp = np.zeros((D_PAD, HID), np.float32)
        wp[:D_IN] = w
        if PAIR_KO4:
            # even mo's ko4 block rows 64:128 carry mo+1's embed weights
            # (consumed by the row-half-64 matmul of the pair)
            for mo in range(0, KO2, 2):
                wp[D_IN:D_IN + EMB, mo * 128:(mo + 1) * 128] = \
                    w[FEAT:D_IN, (mo + 1) * 128:(mo + 2) * 128]
        return wp.reshape(KO1, 128, KO2, 128).transpose(1, 2, 0, 3) \
            .reshape(128, KO1 * HID)

    def pack_w2(w):
        return np.asarray(w, np.float32).reshape(KO2, 128, LIB) \
            .transpose(1, 0, 2).reshape(128, KO2 * LIB)

    nxf = np.asarray(noise_lib_X, np.float64)
    w2x_fused = (np.asarray(Wx2, np.float64) @ nxf).astype(np.float32)
    b2x_fused = (np.asarray(bx2, np.float64) @ nxf).astype(np.float32)
    w1y = np.ascontiguousarray(pack_w1(Wy1).astype(np_a))
    w2y = np.ascontiguousarray(pack_w2(Wy2).astype(np_a))
    blobx = np.ascontiguousarray(np.concatenate(
        [pack_w1(Wx1), pack_w2(w2x_fused)], axis=1).astype(np_a))

    bias = np.zeros((128, 20), np.float32)
    bias[:, 0:8] = np.asarray(by1, np.float32).reshape(8, 128).T
    bias[:, 8:10] = np.asarray(by2, np.float32).reshape(2, 128).T
    bias[:, 10:18] = np.asarray(bx1, np.float32).reshape(8, 128).T
    bias[:, 18:20] = b2x_fused.reshape(2, 128).T

    # ny per core: [128, slot, ko, OUTJ] - slot j carries class cls_of[j, c]
    nyf = np.asarray(noise_lib_Y, np.float32)  # [NCLS, 256, 256]
    ny = np.empty((NCORES, 128, nslots, 2, OUTJ), np.float32)
    for c in range(NCORES):
        sel = nyf[cls_of[:, c]]                       # [nslots, 256, 256]
        ny[c] = sel.reshape(nslots, 2, 128, OUTJ).transpose(2, 0, 1, 3)
    ny = np.ascontiguousarray(ny.reshape(NCORES, 128, -1).astype(np_ny))

    in_maps = []
    for c in range(NCORES):
        m = {"xt_a": xt_a[c], "w1y": w1y, "w2y": w2y,
             "blobx": blobx, "ny": ny[c]}
        if plan["has_bias"]:
            m["bias"] = bias
        if xt_b.shape[-1]:
            m["xt_b"] = xt_b[c]
        in_maps.append(m)
    return in_maps


def _gather_outputs(plan, results):
    core_of, col_of, row_of = (plan["core_of"], plan["col_of"],
                               plan["row_of"])
    nslots = plan["nslots"]
    # slot of each sample from its column
    sstart = np.asarray(plan["sstart"] + [plan["SU_pad"]])
    slot_of = np.searchsorted(sstart, col_of, side="right") - 1
    ox = np.stack([np.asarray(r["outx"]) for r in results])  # [8,128,2*SU]
    oy = np.stack([np.asarray(r["outy"]) for r in results])  # [8,rmax,ns*J]
    ox = ox.reshape(NCORES, 128, 2, plan["SU_pad"])
    oy = oy.reshape(NCORES, plan["rmax"], nslots, OUTJ)
    gx = np.empty((B, 2 * 128), np.float32)
    gx[:, :128] = ox[core_of, :, 0, col_of]
    gx[:, 128:] = ox[core_of, :, 1, col_of]
    gy = oy[core_of, row_of, slot_of].astype(np.float32)
    return gx, gy


def _run(inputs, trace=False):
    plan = _plan(inputs["option"])
    plan["has_bias"] = any(
        np.any(np.asarray(inputs[k])) for k in ("bx1", "bx2", "by1", "by2"))
    nc = _build_nc(plan)
    in_maps = _prepare_inputs(plan, **inputs)
    res = run_bass_kernel_spmd(nc, in_maps, core_ids=list(range(NCORES)),
                               trace=trace)
    gx, gy = _gather_outputs(plan, res.results)
    return (gx, gy), res


def kernel(**inputs):
    (gx, gy), _ = _run(inputs, trace=False)
    return gx, gy


# revision 35
# speedup vs baseline: 1.0554x; 1.0554x over previous
"""Trainium2 Bass kernel for nn_ActorAction (moe_routing).

Computation (see reference):
  option_embed = embed_table[option]              [B, 64]
  all_state    = concat([state, option_embed])    [B, 576]
  cls_X = MLP_relu(all_state; Wx1,bx1,Wx2,bx2)    [B, 256]
  cls_Y = MLP_relu(all_state; Wy1,by1,Wy2,by2)    [B, 256]
  out_X = cls_X @ noise_lib_X                     [B, 256]
  out_Y[b] = cls_Y[b] @ noise_lib_Y[option[b]]    [B, 256]

Strategy: CLASS-sharded routing. Host ranks the 64 classes by count and
forms 8 rank-groups; group j's 8 classes go one-per-core into slot j,
so every core holds 8 whole classes and only needs THEIR noise_lib_Y
rows (1/8 of NY -> 1.05MB vs 8.4MB replicated). The Bass program is
shared SPMD: slot capacities c_j = max count in rank group j are
compile-time constants; per-core inputs (xt columns, ny slot payload)
differ. MLPs run feature-major (weights stationary, samples streamed);
routing is ONE psum-wide matmul pair per slot (c_j <= 128 rows), 16
matmuls total instead of 128 32-row tile_position matmuls. Outputs are
written bf16 to halve output DMA. Input DMAs are split and ordered by
first consumer; a stream of tiny dummy matmuls warms the PE (HAM
un-throttle) before real data lands.
"""
import os
from contextlib import ExitStack

import numpy as np
import ml_dtypes

import concourse.bacc as bacc
import concourse.mybir as mybir
import concourse.tile as tile
from concourse.bass_utils import run_bass_kernel_spmd

F32 = mybir.dt.float32
F32R = mybir.dt.float32r
BF16 = mybir.dt.bfloat16
AFT = mybir.ActivationFunctionType

# problem dims (hardcoded per spec)
B, FEAT, EMB, HID, NCLS = 4096, 512, 64, 1024, 64
LIB = 256          # LIB_X == LIB_Y
OUTJ = 256
NCORES = 8
D_IN = FEAT + EMB          # 576
KO1 = 5                    # ceil(576/128) K-blocks for layer 1
D_PAD = KO1 * 128          # 640
KO2 = HID // 128           # 8
N_WARMUP = 34              # dummy matmuls (N=128) to warm PE during loads

_DT_MAP = {"f32": F32, "f32r": F32R, "bf16": BF16}
_NP_MAP = {"f32": np.float32, "f32r": np.float32, "bf16": ml_dtypes.bfloat16}
DT_A_NAME = os.environ.get("KDT_A", "bf16")    # MLP weights/acts + NX path
DT_NY_NAME = os.environ.get("KDT_NY", "bf16")  # noise_lib_Y + cls_Y path
# ko4 (embed block) has only 64 live K rows: run two mo-chains' ko4 as
# concurrent K=64 matmuls in the two row-halves of the PE array
PAIR_KO4 = os.environ.get("KPAIR", "0") == "1"


def _round_up(a, b):
    return (a + b - 1) // b * b


def _plan(option):
    opt = np.asarray(option).astype(np.int64).ravel()
    assert opt.shape[0] == B
    g = np.bincount(opt, minlength=NCLS)
    order = np.argsort(opt, kind="stable")
    starts = np.concatenate([[0], np.cumsum(g)])
    # pseudo-classes: (class_id, sample idx array), each <= 128 samples
    pieces = []
    for m in range(NCLS):
        idx = order[starts[m]:starts[m + 1]]
        if len(idx) == 0:
            pieces.append((m, idx))
        for o in range(0, len(idx), 128):
            pieces.append((m, idx[o:o + 128]))
    pieces.sort(key=lambda t: -len(t[1]))
    while len(pieces) % NCORES:
        pieces.append((pieces[-1][0], np.empty(0, np.int64)))
    nslots = len(pieces) // NCORES

    caps = []                      # per-slot capacity (compile-time)
    sstart = []                    # per-slot column start
    cls_of = np.zeros((nslots, NCORES), np.int64)  # class in (slot, core)
    core_of = np.empty(B, np.int64)
    col_of = np.empty(B, np.int64)
    row_of = np.empty(B, np.int64)
    s = 0
    for j in range(nslots):
        grp = pieces[NCORES * j:NCORES * (j + 1)]
        cap = max(4, _round_up(len(grp[0][1]), 4))
        for c, (m, idx) in enumerate(grp):
            cls_of[j, c] = m
            if len(idx):
                core_of[idx] = c
                col_of[idx] = s + np.arange(len(idx))
                row_of[idx] = np.arange(len(idx))
        sstart.append(s)
        caps.append(cap)
        s += cap
    SU_pad = s
    rmax = max(caps)

    # column chunks (psum free dim <= 512), boundaries at slot starts
    chunks = []
    c0 = 0
    for j in range(nslots):
        if sstart[j] + caps[j] - c0 > 512:
            chunks.append((c0, sstart[j]))
            c0 = sstart[j]
    chunks.append((c0, SU_pad))
    assert all(c1 - c0 <= 512 for c0, c1 in chunks)
    assert rmax <= 128

    return dict(opt=opt, caps=caps, sstart=sstart, nslots=nslots,
                SU_pad=SU_pad, rmax=rmax, chunks=chunks, cls_of=cls_of,
                core_of=core_of, col_of=col_of, row_of=row_of,
                has_bias=True)


_NC_CACHE = {}


def _build_nc(plan):
    DT_A = _DT_MAP[DT_A_NAME]
    DT_NY = _DT_MAP[DT_NY_NAME]
    SU_pad = plan["SU_pad"]
    chunks = plan["chunks"]
    caps = plan["caps"]
    sstart = plan["sstart"]
    nslots = plan["nslots"]
    rmax = plan["rmax"]
    has_bias = plan["has_bias"]

    key = (tuple(caps), tuple(chunks), has_bias, PAIR_KO4,
           DT_A_NAME, DT_NY_NAME)
    if key in _NC_CACHE:
        return _NC_CACHE[key]

    c_w1 = KO1 * HID
    c_w2 = KO2 * LIB
    c_bx = c_w1 + c_w2                 # X-branch blob columns
    NY_COLS = nslots * 2 * OUTJ

    ch0 = chunks[0][1]
    c_xt_a = KO1 * ch0
    c_xt_b = KO1 * SU_pad - c_xt_a
    c_mo = KO1 * 128           # one mo-block of W1

    nc = bacc.Bacc()
    xt_a_d = nc.dram_tensor("xt_a", [128, c_xt_a], DT_A, kind="ExternalInput")
    xt_b_d = (nc.dram_tensor("xt_b", [128, c_xt_b], DT_A, kind="ExternalInput")
              if c_xt_b else None)
    w1y_d = nc.dram_tensor("w1y", [128, c_w1], DT_A, kind="ExternalInput")
    # w1y piece boundaries (in mo blocks): mo0 | mo1 | mo2-3 | mo4-7
    w1y_pieces = [(0, 1), (1, 2), (2, 4), (4, KO2)]
    w2y_d = nc.dram_tensor("w2y", [128, c_w2], DT_A, kind="ExternalInput")
    blobx_d = nc.dram_tensor("blobx", [128, c_bx], DT_A, kind="ExternalInput")
    blobx_pieces = [(0, 4 * c_mo), (4 * c_mo, c_bx)]  # w1x mo0-3 | rest
    bias_d = (nc.dram_tensor("bias", [128, 20], F32, kind="ExternalInput")
              if has_bias else None)
    ny_d = nc.dram_tensor("ny", [128, NY_COLS], DT_NY, kind="ExternalInput")
    outx_d = nc.dram_tensor("outx", [128, 2 * SU_pad], BF16,
                            kind="ExternalOutput")
    outy_d = nc.dram_tensor("outy", [rmax, nslots * OUTJ], BF16,
                            kind="ExternalOutput")

    with tile.TileContext(nc) as tc, ExitStack() as ctx:
        const = ctx.enter_context(tc.tile_pool(name="const", bufs=1))
        act = ctx.enter_context(tc.tile_pool(name="act", bufs=1))
        hpool = ctx.enter_context(tc.tile_pool(name="hpool", bufs=2))
        mlp_ps = ctx.enter_context(tc.tile_pool(name="mlp_ps", bufs=5, space="PSUM"))
        rt_ps = ctx.enter_context(tc.tile_pool(name="rt_ps", bufs=3, space="PSUM"))

        # input DMAs: few coarse dma_starts, ordered by first consumer.
        # Each dma_start costs ~0.65us of serialized trigger time on the
        # sync sequencer, and all inputs stream through ONE HW queue in
        # trigger order at ~300GB/s, first data ~1.5us after trigger #1.
        # So the order below is tuned to the PE's weight consumption.
        w1_tiles = {"y": [None] * KO2, "x": None}
        w1y_sbs = {}
        for lo, hi in w1y_pieces:
            t = const.tile([128, (hi - lo) * c_mo], DT_A, tag=f"w1y{lo}",
                           name=f"w1y{lo}")
            w1y_sbs[lo] = t
            v = t.rearrange("p (mo ko m) -> p mo ko m", mo=hi - lo, ko=KO1)
            for mo in range(lo, hi):
                w1_tiles["y"][mo] = v[:, mo - lo]
        nc.sync.dma_start(w1y_sbs[0][:], w1y_d[:, 0:c_mo])
        xt_a_sb = const.tile([128, c_xt_a], DT_A, tag="xta", name="xta")
        nc.sync.dma_start(xt_a_sb[:], xt_a_d[:])
        xt_a_v = xt_a_sb.rearrange("p (ko b) -> p ko b", ko=KO1)
        for lo, hi in w1y_pieces[1:]:
            # emitted after xt_a: each piece arrives just ahead of its
            # mo chain's start (the stream is the binding resource)
            nc.sync.dma_start(w1y_sbs[lo][:], w1y_d[:, lo * c_mo:hi * c_mo])
        if c_xt_b:
            xt_b_sb = const.tile([128, c_xt_b], DT_A)
            nc.sync.dma_start(xt_b_sb[:], xt_b_d[:])
        w2y_sb = const.tile([128, c_w2], DT_A)
        nc.sync.dma_start(w2y_sb[:], w2y_d[:])
        if has_bias:
            bias_sb = const.tile([128, 20], F32)
            nc.sync.dma_start(bias_sb[:], bias_d[:])
        ny_sb = const.tile([128, NY_COLS], DT_NY)
        nc.sync.dma_start(ny_sb[:], ny_d[:])
        ny_v = ny_sb.rearrange("p (s ko j) -> p s ko j", s=nslots, ko=2)
        blobx_sb = const.tile([128, c_bx], DT_A)
        for lo, hi in blobx_pieces:
            nc.sync.dma_start(blobx_sb[:, lo:hi], blobx_d[:, lo:hi])
        w1_tiles["x"] = blobx_sb[:, 0:c_w1].rearrange(
            "p (mo ko m) -> p mo ko m", mo=KO2, ko=KO1)
        w2x_sb = blobx_sb[:, c_w1:c_w1 + c_w2]

        # PE warmup: dummy matmuls on an uninitialized tile (values are
        # irrelevant, the psum result is never read) keep the PE busy from
        # right after the preamble so HAM un-throttles before real work.
        warm_sb = const.tile([128, 128], BF16)
        nc.any.memset(warm_sb[:], 0)
        wups = rt_ps.tile([40, 128], F32, tag="rt", name="rt_ps_t")
        for _ in range(N_WARMUP):
            nc.tensor.matmul(wups[:], lhsT=warm_sb[:, :40], rhs=warm_sb[:],
                             start=True, stop=True)

        def xtv(ci, ko, cw):
            if ci == 1:
                return xt_b_sb.rearrange("p (ko b) -> p ko b", ko=KO1)[:, ko, :cw]
            return xt_a_v[:, ko, :cw]

        def w1v(br, mo, ko, msl):
            if br == "y":
                return w1_tiles["y"][mo][:, ko, msl]
            return w1_tiles["x"][:, mo, ko, msl]

        w2_v = {"y": w2y_sb.rearrange("p (ko m) -> p ko m", ko=KO2),
                "x": w2x_sb.rearrange("p (ko m) -> p ko m", ko=KO2)}
        # bias cols: b1y[0:8] b2y[8:10] b1x[10:18] b2x[18:20]
        bcol = {"y": (0, 8), "x": (10, 18)}

        outy_sb = act.tile([128, nslots, OUTJ], BF16, tag="outy")
        outy_dv = outy_d.rearrange("p (s j) -> p s j", s=nslots)
        outxT = act.tile([128, 2, SU_pad], BF16, tag="outxT")
        outx_dv = outx_d.rearrange("p (jo b) -> p jo b", jo=2)
        clsy = act.tile([128, 2, SU_pad], DT_NY, tag="clsy", name="clsy")
        CH_MAX = max(c1 - c0 for c0, c1 in chunks)

        def post_op(idx, out, ps, func, bias_ap):
            # alternate ScalarE/VectorE per index: splits the psum-drain
            # load across both engines so neither gates the PE.
            if idx % 2 == 0:
                if not has_bias and func is AFT.Identity:
                    nc.scalar.copy(out, ps)
                else:
                    nc.scalar.activation(out, ps, func,
                                         bias=bias_ap if has_bias else 0.0)
            elif func is AFT.Relu:
                if has_bias:
                    nc.vector.tensor_scalar(out, ps, bias_ap, 0.0,
                                            mybir.AluOpType.add,
                                            mybir.AluOpType.max)
                else:
                    nc.vector.tensor_scalar(out, ps, 0.0, None,
                                            mybir.AluOpType.max)
            elif has_bias:
                nc.vector.tensor_scalar(out, ps, bias_ap, None,
                                        mybir.AluOpType.add)
            else:
                nc.vector.tensor_copy(out, ps)

        def mlp(br, after_chunk=None):
            # chunk-major: both layers of chunk ci complete before ci+1,
            # so per-chunk consumers (routing / outx DMA) start early.
            h_sb = hpool.tile([128, KO2, SU_pad], DT_A, tag="h", name=f"h_{br}")
            b1o, b2o = bcol[br]
            for ci, (c0, c1) in enumerate(chunks):
                cw = c1 - c0
                if PAIR_KO4:
                    for t in range(KO2 // 2):
                        moa, mob = 2 * t, 2 * t + 1
                        pss = []
                        for mo in (moa, mob):
                            ps = mlp_ps.tile([128, CH_MAX], F32, tag="mlp",
                                             name="mlp_ps_t")[:, :cw]
                            pss.append(ps)
                            for ko in range(KO1 - 1):
                                nc.tensor.matmul(
                                    ps, lhsT=w1v(br, mo, ko, slice(0, 128)),
                                    rhs=xtv(ci, ko, cw),
                                    start=(ko == 0), stop=False)
                        # ko4: both mo chains' embed blocks concurrently in
                        # the two row-halves of the PE (weights for mob are
                        # packed into rows 64:128 of moa's ko4 block)
                        wa = w1v(br, moa, KO1 - 1, slice(0, 128))
                        xr = xtv(ci, KO1 - 1, cw)
                        for hi, ps in enumerate(pss):
                            h0 = 64 * hi
                            nc.tensor.matmul(
                                ps, lhsT=wa[h0:h0 + 64, :],
                                rhs=xr[h0:h0 + 64, :],
                                start=False, stop=True,
                                tile_position=(h0, 0))
                        for hi, mo in enumerate((moa, mob)):
                            post_op(mo, h_sb[:, mo, c0:c1], pss[hi], AFT.Relu,
                                    bias_sb[:, b1o + mo:b1o + mo + 1]
                                    if has_bias else None)
                else:
                    for mo in range(KO2):
                        ps = mlp_ps.tile([128, CH_MAX], F32, tag="mlp",
                                         name="mlp_ps_t")[:, :cw]
                        for ko in range(KO1):
                            nc.tensor.matmul(
                                ps, lhsT=w1v(br, mo, ko, slice(0, 128)),
                                rhs=xtv(ci, ko, cw),
                                start=(ko == 0), stop=(ko == KO1 - 1))
                        post_op(mo, h_sb[:, mo, c0:c1], ps, AFT.Relu,
                                bias_sb[:, b1o + mo:b1o + mo + 1]
                                if has_bias else None)
                for jo in range(2):
                    ps = mlp_ps.tile([128, CH_MAX], F32, tag="mlp",
                                     name="mlp_ps_t")[:, :cw]
                    for ko in range(KO2):
                        nc.tensor.matmul(
                            ps, lhsT=w2_v[br][:, ko, jo * 128:(jo + 1) * 128],
                            rhs=h_sb[:, ko, c0:c1],
                            start=(ko == 0), stop=(ko == KO2 - 1))
                    tgt = clsy if br == "y" else outxT
                    post_op(jo, tgt[:, jo, c0:c1], ps, AFT.Identity,
                            bias_sb[:, b2o + jo:b2o + jo + 1]
                            if has_bias else None)
                if br == "x":
                    nc.sync.dma_start(outx_dv[:, :, c0:c1],
                                      outxT[:, :, c0:c1])
                elif after_chunk is not None:
                    after_chunk(ci, c1)

        # routing: slot j holds one whole class (per core); out_Y rows =
        # clsY[:, s:s+c].T @ NY[slot j]  -- one full-width matmul pair.
        # Emitted per chunk as soon as that chunk's clsY is ready; copies
        # alternate ScalarE/VectorE so psum drains don't gate the PE.
        rt_done = [0]

        def route_upto(ci, c1):
            last = ci == len(chunks) - 1
            j0 = rt_done[0]
            j = j0
            while j < nslots and (last or sstart[j] + caps[j] <= c1):
                ps = rt_ps.tile([128, OUTJ], F32, tag="rt", name="rt_ps_t")
                for ko in range(2):
                    nc.tensor.matmul(
                        ps[0:caps[j], :],
                        lhsT=clsy[:, ko, sstart[j]:sstart[j] + caps[j]],
                        rhs=ny_v[:, j, ko, :],
                        start=(ko == 0), stop=(ko == 1))
                if j % 2 == 0:
                    nc.scalar.copy(outy_sb[:, j, :], ps[:])
                else:
                    nc.vector.tensor_copy(outy_sb[:, j, :], ps[:])
                j += 1
            rt_done[0] = j
            if j > j0:
                nc.sync.dma_start(outy_dv[:, j0:j, :],
                                  outy_sb[0:rmax, j0:j, :])

        mlp("y", after_chunk=route_upto)
        mlp("x")   # out_X comes straight from the fused MLP2-X (DMA inside)

    nc.compile()
    _NC_CACHE[key] = nc
    return nc


def _prepare_inputs(plan, state, option, embed_table, Wx1, bx1, Wx2, bx2,
                    Wy1, by1, Wy2, by2, noise_lib_X, noise_lib_Y):
    np_a = _NP_MAP[DT_A_NAME]
    np_ny = _NP_MAP[DT_NY_NAME]
    SU_pad = plan["SU_pad"]
    opt = plan["opt"]
    nslots = plan["nslots"]
    core_of, col_of = plan["core_of"], plan["col_of"]
    cls_of = plan["cls_of"]

    state = np.asarray(state, np.float32)
    embed_table = np.asarray(embed_table, np.float32)

    # per-core feature-major inputs
    Xall = np.zeros((NCORES, SU_pad, D_PAD), np.float32)
    Xall[core_of, col_of, :FEAT] = state
    Xall[core_of, col_of, FEAT:D_IN] = embed_table[opt]
    if PAIR_KO4:
        # duplicate embed rows into the ko4 zero-pad so the two row-half
        # K=64 matmuls (mo pair) both see the embed features
        Xall[core_of, col_of, D_IN:D_IN + EMB] = embed_table[opt]
    # [NCORES, 128, KO1, SU_pad]
    xt = Xall.transpose(0, 2, 1).reshape(NCORES, KO1, 128, SU_pad) \
        .transpose(0, 2, 1, 3).astype(np_a)
    ch0 = plan["chunks"][0][1]
    xt_a = np.ascontiguousarray(xt[:, :, :, :ch0]).reshape(NCORES, 128, -1)
    xt_b = np.ascontiguousarray(xt[:, :, :, ch0:]).reshape(NCORES, 128, -1)

    def pack_w1(w):
        # mo-major: [128p, mo, ko, 128] flattened
        w = np.asarray(w, np.float32)
        wp = np.zeros((D_PAD, HID), np.float32)
        wp[:D_IN] = w
        if PAIR_KO4:
            # even mo's ko4 block rows 64:128 carry mo+1's embed weights
            # (consumed by the row-half-64 matmul of the pair)
            for mo in range(0, KO2, 2):
                wp[D_IN:D_IN + EMB, mo * 128:(mo + 1) * 128] = \
                    w[FEAT:D_IN, (mo + 1) * 128:(mo + 2) * 128]
        return wp.reshape(KO1, 128, KO2, 128).transpose(1, 2, 0, 3) \
            .reshape(128, KO1 * HID)

    def pack_w2(w):
        return np.asarray(w, np.float32).reshape(KO2, 128, LIB) \
            .transpose(1, 0, 2).reshape(128, KO2 * LIB)

    nxf = np.asarray(noise_lib_X, np.float64)
    w2x_fused = (np.asarray(Wx2, np.float64) @ nxf).astype(np.float32)
    b2x_fused = (np.asarray(bx2, np.float64) @ nxf).astype(np.float32)
    w1y = np.ascontiguousarray(pack_w1(Wy1).astype(np_a))
    w2y = np.ascontiguousarray(pack_w2(Wy2).astype(np_a))
    blobx = np.ascontiguousarray(np.concatenate(
        [pack_w1(Wx1), pack_w2(w2x_fused)], axis=1).astype(np_a))

    bias = np.zeros((128, 20), np.float32)
    bias[:, 0:8] = np.asarray(by1, np.float32).reshape(8, 128).T
    bias[:, 8:10] = np.asarray(by2, np.float32).reshape(2, 128).T
    bias[:, 10:18] = np.asarray(bx1, np.float32).reshape(8, 128).T
    bias[:, 18:20] = b2x_fused.reshape(2, 128).T

    # ny per core: [128, slot, ko, OUTJ] - slot j carries class cls_of[j, c]
    nyf = np.asarray(noise_lib_Y, np.float32)  # [NCLS, 256, 256]
    ny = np.empty((NCORES, 128, nslots, 2, OUTJ), np.float32)
    for c in range(NCORES):
        sel = nyf[cls_of[:, c]]                       # [nslots, 256, 256]
        ny[c] = sel.reshape(nslots, 2, 128, OUTJ).transpose(2, 0, 1, 3)
    ny = np.ascontiguousarray(ny.reshape(NCORES, 128, -1).astype(np_ny))

    in_maps = []
    for c in range(NCORES):
        m = {"xt_a": xt_a[c], "w1y": w1y, "w2y": w2y,
             "blobx": blobx, "ny": ny[c]}
        if plan["has_bias"]:
            m["bias"] = bias
        if xt_b.shape[-1]:
            m["xt_b"] = xt_b[c]
        in_maps.append(m)
    return in_maps


def _gather_outputs(plan, results):
    core_of, col_of, row_of = (plan["core_of"], plan["col_of"],
                               plan["row_of"])
    nslots = plan["nslots"]
    # slot of each sample from its column
    sstart = np.asarray(plan["sstart"] + [plan["SU_pad"]])
    slot_of = np.searchsorted(sstart, col_of, side="right") - 1
    ox = np.stack([np.asarray(r["outx"]) for r in results])  # [8,128,2*SU]
    oy = np.stack([np.asarray(r["outy"]) for r in results])  # [8,rmax,ns*J]
    ox = ox.reshape(NCORES, 128, 2, plan["SU_pad"])
    oy = oy.reshape(NCORES, plan["rmax"], nslots, OUTJ)
    gx = np.empty((B, 2 * 128), np.float32)
    gx[:, :128] = ox[core_of, :, 0, col_of]
    gx[:, 128:] = ox[core_of, :, 1, col_of]
    gy = oy[core_of, row_of, slot_of].astype(np.float32)
    return gx, gy


def _run(inputs, trace=False):
    plan = _plan(inputs["option"])
    plan["has_bias"] = any(
        np.any(np.asarray(inputs[k])) for k in ("bx1", "bx2", "by1", "by2"))
    nc = _build_nc(plan)
    in_maps = _prepare_inputs(plan, **inputs)
    res = run_bass_kernel_spmd(nc, in_maps, core_ids=list(range(NCORES)),
                               trace=trace)
    gx, gy = _gather_outputs(plan, res.results)
    return (gx, gy), res


def kernel(**inputs):
    (gx, gy), _ = _run(inputs, trace=False)
    return gx, gy
